# revision 2
# baseline (speedup 1.0000x reference)
"""BertSelfAttention Trainium2 kernel.

Shapes: hidden_states [S=1024, B=4, D=1024], H=16 heads of DH=64.
Sharding: 2 heads per core (8 cores). Each core receives the full hidden
states (pre-transposed + bf16-cast on host) and a 128-row slice of each
projection weight, computes the full attention chain for its two heads with
no cross-core communication, and writes unnormalized ctx^T plus the softmax
denominator per (batch, head); the final division happens on host in fp64.

Device-side layout tricks:
  - scores are computed transposed (scoresT[u, t] = q_t . k_u) so the
    additive attention mask (per key position u) is a per-partition bias
    that fuses into the Exp activation: probsT = exp(scores/8 + mask).
    Both heads' K=64 score matmuls row-pack into the 128-row PE array
    (row groups 0-63 / 64-127) and run concurrently; one wide ACT Exp
    per (batch, u-tile) covers both heads ([128, 2048]) to amortize the
    ~352-cycle ACT instruction overhead.
  - V carries a prepended ones-column, so the AV matmul produces the
    softmax denominator in row 0 of ctxT for free.
  - inputs are host-relaid so every DMA has large contiguous rows; the
    bulk hidden-state load is split across both HWDGE queues (SP / ACT)
    with batch-0 pieces first so the projection prologue chases the DMA.
"""

import os
import numpy as np
import ml_dtypes

S, B, D, H = 1024, 4, 1024, 16
DH = D // H          # 64
NCORES = 8
HPC = H // NCORES    # heads per core = 2
P = 128              # partitions / d-tile / u-tile
DCH = D // P         # 8 contraction tiles
BS = B * S           # 4096 flattened (b, s)
CH = 512             # matmul free-dim chunk (PSUM bank limit for fp32)

_compiled_nc = None
last_exec_time_ns = None
last_results = None


def _build():
    import concourse.bacc as bacc
    import concourse.mybir as mybir
    import concourse.tile as tile
    from contextlib import ExitStack

    f32 = mybir.dt.float32
    bf16 = mybir.dt.bfloat16
    AF = mybir.ActivationFunctionType

    nc = bacc.Bacc("TRN2", target_bir_lowering=False, debug=False,
                   num_devices=NCORES)

    # host-relaid inputs: hT [q, p, dc, s], weights [p, dc, m]
    hT_d = nc.dram_tensor("hT", [B, P, DCH, S], bf16, kind="ExternalInput")
    wq_d = nc.dram_tensor("wq", [P, DCH, P], bf16, kind="ExternalInput")
    wk_d = nc.dram_tensor("wk", [P, DCH, P], bf16, kind="ExternalInput")
    wv_d = nc.dram_tensor("wv", [P, DCH, P], bf16, kind="ExternalInput")
    # packed per-partition constants: [bq | bk | bvb(128) | maskT(8*4)]
    misc_d = nc.dram_tensor("misc", [P, 2 + P + DCH * B], f32,
                            kind="ExternalInput")
    # row 0 of the DH+1 dim is the softmax denominator (normalized on host)
    out_d = nc.dram_tensor("out", [B, HPC, DH + 1, S], f32,
                           kind="ExternalOutput")

    with tile.TileContext(nc) as tc, ExitStack() as ctx:
        persist = ctx.enter_context(tc.tile_pool(name="persist", bufs=1))
        probs_pool = ctx.enter_context(tc.tile_pool(name="probs", bufs=17))
        out_pool = ctx.enter_context(tc.tile_pool(name="outp", bufs=4))
        ps_sc = ctx.enter_context(tc.tile_pool(name="ps_sc", bufs=1, space="PSUM"))
        ps_mm = ctx.enter_context(tc.tile_pool(name="ps_mm", bufs=2, space="PSUM"))
        ps_ctx = ctx.enter_context(tc.tile_pool(name="ps_ctx", bufs=2, space="PSUM"))

        # ---- persistent SBUF tensors ----
        hT_sb = persist.tile([P, DCH, BS], bf16)        # hidden^T, d-tiled
        wq_sb = persist.tile([P, DCH, P], bf16)
        wk_sb = persist.tile([P, DCH, P], bf16)
        wv_sb = persist.tile([P, DCH, P], bf16)
        misc_sb = persist.tile([P, 2 + P + DCH * B], f32)
        qT_sb = persist.tile([P, BS], bf16)             # Q^T [i, t]
        kT_sb = persist.tile([P, BS], bf16)             # K^T [i, t]
        # V in [t, j] layout + ones column per head: [t-part, t-tile, head, DH+1]
        v_sb = persist.tile([P, BS // P, HPC, DH + 1], bf16)
        dummy_sb = persist.tile([P, CH], bf16)

        bq_sb = misc_sb[:, 0:1]
        bk_sb = misc_sb[:, 1:2]
        bvb_sb = misc_sb[:, 2:2 + P]

        def mask_bias(uc, bi):
            c = 2 + P + uc * B + bi
            return misc_sb[:, c:c + 1]

        # ---- HAM warmup: dead matmuls keep the PE busy while inputs load,
        # so the real work starts at the 2.4 GHz clock.
        nc.gpsimd.memset(dummy_sb[:], 0.0)
        for _ in range(9):
            d_ps = ps_mm.tile([P, CH], f32, tag="mm", name="d_ps")
            nc.tensor.matmul(d_ps[:], dummy_sb[:, 0:P], dummy_sb[:],
                             start=True, stop=True)

        # ---- input DMAs ----
        # Two HWDGE queues (SP + ACT). Batch 0 pieces first (dc-pairs split
        # across both queues) so the projection prologue can chase the DMA;
        # remaining batches land as single 2MB transfers for bandwidth.
        def hT_piece(q, dc0, ndc, eng):
            qsl = slice(q * S, (q + 1) * S)
            eng.dma_start(hT_sb[:, dc0:dc0 + ndc, qsl],
                          hT_d.ap()[q, :, dc0:dc0 + ndc, :])

        nc.sync.dma_start(wk_sb[:], wk_d.ap())
        nc.scalar.dma_start(wq_sb[:], wq_d.ap())
        hT_piece(0, 0, 2, nc.sync)
        hT_piece(0, 4, 2, nc.scalar)
        hT_piece(0, 2, 2, nc.sync)
        hT_piece(0, 6, 2, nc.scalar)
        nc.sync.dma_start(misc_sb[:], misc_d.ap())
        nc.sync.dma_start(wv_sb[:], wv_d.ap())
        hT_piece(1, 0, DCH, nc.scalar)
        hT_piece(2, 0, DCH, nc.sync)
        hT_piece(3, 0, DCH, nc.scalar)

        nc.vector.memset(v_sb[:, :, :, 0:1], 1.0)

        scale = 1.0 / float(np.sqrt(DH))

        def emit_qk_chunk(w_sb, b_sb, dst, ci):
            sl = slice(ci * CH, (ci + 1) * CH)
            qk_ps = ps_mm.tile([P, CH], f32, tag="mm", name="qk_ps")
            for dc in range(DCH):
                nc.tensor.matmul(
                    qk_ps[:], w_sb[:, dc, :], hT_sb[:, dc, sl],
                    start=(dc == 0), stop=(dc == DCH - 1))
            nc.vector.tensor_scalar_add(dst[:, sl], qk_ps[:], b_sb[:])

        def emit_v_tile(tt):
            tsl = slice(tt * P, (tt + 1) * P)
            v_ps = ps_mm.tile([P, P], f32, tag="mm", name="v_ps")
            for dc in range(DCH):
                nc.tensor.matmul(
                    v_ps[:], hT_sb[:, dc, tsl], wv_sb[:, dc, :],
                    start=(dc == 0), stop=(dc == DCH - 1))
            nc.vector.tensor_add(
                v_sb[:, tt, 0:HPC, 1:DH + 1],
                v_ps[:].rearrange("p (h j) -> p h j", j=DH),
                bvb_sb[:].rearrange("p (h j) -> p h j", j=DH))

        def qk_thunks(bi):
            th = []
            for w_sb, b_sb, dst in ((wq_sb, bq_sb, qT_sb), (wk_sb, bk_sb, kT_sb)):
                for ci in range(2 * bi, 2 * bi + 2):
                    th.append(lambda w=w_sb, b=b_sb, d=dst, c=ci:
                              emit_qk_chunk(w, b, d, c))
            return th

        def v_thunks(bi):
            return [lambda t=tt: emit_v_tile(t)
                    for tt in range(8 * bi, 8 * bi + 8)]

        def emit_av_mm(bi, hl, pps, ctx_tiles, uc):
            for c2 in range(2):
                nc.tensor.matmul(
                    ctx_tiles[c2][:],
                    v_sb[:, bi * 8 + uc, hl, :],
                    pps[uc][:, hl, c2 * CH:(c2 + 1) * CH],
                    start=(uc == 0), stop=(uc == DCH - 1))

        def emit_out(bi, hl, ctx_tiles):
            # ctx_ps row 0 = denominator, rows 1..DH = unnormalized ctx^T.
            # Ship both; host does the division.
            for c2 in range(2):
                csl = slice(c2 * CH, (c2 + 1) * CH)
                o_sb = out_pool.tile([DH + 1, CH], f32, name="o_sb")
                nc.vector.tensor_copy(o_sb[:], ctx_tiles[c2][:])
                eng = nc.sync if (hl + c2) % 2 == 0 else nc.scalar
                eng.dma_start(out_d.ap()[bi, hl, :, csl], o_sb[:])

        def new_ctx_tiles(pool=None, tag="ctx"):
            pool = pool or ps_ctx
            return [pool.tile([DH + 1, CH], f32, tag=tag, name="ctx_ps")
                    for _ in range(2)]

        # ---- prologue: batch 0's Q/K projections, dc-major so the four
        # PSUM accumulation groups chase the arriving hT pieces. Group
        # order (K c0, Q c0, Q c1, K c1) lets the first score tiles fire
        # before the last group completes.
        pro_specs = [(wk_sb, bk_sb, kT_sb, 0), (wq_sb, bq_sb, qT_sb, 0),
                     (wq_sb, bq_sb, qT_sb, 1), (wk_sb, bk_sb, kT_sb, 1)]
        pro = ps_sc.tile([P, HPC, S], f32, tag="sc", name="pro")
        pro_flat = pro.rearrange("p h s -> p (h s)")
        for dc in range(DCH):
            for g, (w_sb, b_sb, dst, ci) in enumerate(pro_specs):
                nc.tensor.matmul(
                    pro_flat[:, g * CH:(g + 1) * CH], w_sb[:, dc, :],
                    hT_sb[:, dc, ci * CH:(ci + 1) * CH],
                    start=(dc == 0), stop=(dc == DCH - 1))
        for g, (w_sb, b_sb, dst, ci) in enumerate(pro_specs):
            osl = slice(ci * CH, (ci + 1) * CH)
            src = pro_flat[:, g * CH:(g + 1) * CH]
            if g < 2:      # split the bias-adds across DVE and ACT
                nc.vector.tensor_scalar_add(dst[:, osl], src, b_sb[:])
            else:
                nc.scalar.activation(dst[:, osl], src, AF.Identity,
                                     bias=b_sb[:])

        # ---- software pipeline over batches. Per batch: scores+exp for
        # both heads per u-tile, AV of the previous batch woven in, plus
        # this batch's V projection and the next batch's Q/K projection
        # to fill the PE slack.
        prev = None          # (bi, [pp2 tiles per uc])
        queue = []
        for bi in range(B):
            while queue:
                queue.pop(0)()
            queue = v_thunks(bi)
            if bi + 1 < B:
                queue += qk_thunks(bi + 1)
            pps = []
            ctxA = new_ctx_tiles() if prev is not None else None
            ctxB = None
            # h1 AV of the previous batch woven into steps 2-4 (its ctx
            # tiles borrow mm-pool slots; finishing by step 4 releases them
            # before the late projection thunks need them)
            h1_sched = {2: (0, 3), 3: (3, 6), 4: (6, 8)}
            for uc in range(DCH):
                usl = slice(bi * S + uc * P, bi * S + (uc + 1) * P)
                sc2 = ps_sc.tile([P, HPC, S], f32, tag="sc", name="sc2")
                for c2 in range(2):
                    qsl = slice(bi * S + c2 * CH, bi * S + (c2 + 1) * CH)
                    csl = slice(c2 * CH, (c2 + 1) * CH)
                    for hl in range(HPC):
                        hsl = slice(hl * DH, (hl + 1) * DH)
                        nc.tensor.matmul(
                            sc2[:, hl, csl],
                            kT_sb[hsl, usl], qT_sb[hsl, qsl],
                            start=True, stop=True)
                pp2 = probs_pool.tile([P, HPC, S], bf16, name="pp2")
                nc.scalar.activation(
                    pp2.rearrange("p h s -> p (h s)"),
                    sc2.rearrange("p h s -> p (h s)"),
                    AF.Exp, bias=mask_bias(uc, bi), scale=scale)
                pps.append(pp2)
                if prev is not None:
                    emit_av_mm(prev[0], 0, prev[1], ctxA, uc)
                    if uc in h1_sched:
                        if ctxB is None:
                            ctxB = new_ctx_tiles(ps_mm, "mm")
                        for u2 in range(*h1_sched[uc]):
                            emit_av_mm(prev[0], 1, prev[1], ctxB, u2)
                        if h1_sched[uc][1] == DCH:
                            emit_out(prev[0], 1, ctxB)
                for _ in range(2 if uc < 4 else 1):
                    if queue:
                        queue.pop(0)()
            if prev is not None:
                emit_out(prev[0], 0, ctxA)
            prev = (bi, pps)
        # epilogue: last batch's attention output
        while queue:
            queue.pop(0)()
        pbi, ppps = prev
        ctxE0 = new_ctx_tiles()
        for uc in range(DCH):
            emit_av_mm(pbi, 0, ppps, ctxE0, uc)
        ctxE1 = new_ctx_tiles(ps_mm, "mm")
        for uc in range(DCH):
            emit_av_mm(pbi, 1, ppps, ctxE1, uc)
        emit_out(pbi, 0, ctxE0)
        emit_out(pbi, 1, ctxE1)

    nc.compile()
    return nc


def _get_nc():
    global _compiled_nc
    if _compiled_nc is None:
        _compiled_nc = _build()
    return _compiled_nc


def prepare_in_maps(hidden_states, attention_mask, Wq, bq, Wk, bk, Wv, bv):
    bf16 = ml_dtypes.bfloat16

    hs = np.asarray(hidden_states, dtype=np.float32)            # [S, B, D]
    # hT5[q, p, dc, s] = hs[s, q, dc*128 + p]
    hT5 = np.ascontiguousarray(
        hs.transpose(1, 2, 0).reshape(B, DCH, P, S).transpose(0, 2, 1, 3)
    ).astype(bf16)
    maskT = np.ascontiguousarray(
        np.asarray(attention_mask, dtype=np.float32).reshape(B, S).T)
    Wq = np.asarray(Wq, dtype=np.float32)
    Wk = np.asarray(Wk, dtype=np.float32)
    Wv = np.asarray(Wv, dtype=np.float32)
    bq = np.asarray(bq, dtype=np.float32)
    bk = np.asarray(bk, dtype=np.float32)
    bv = np.asarray(bv, dtype=np.float32)

    def w_pack(W, sl):
        # [p, dc, m] with element = W[sl][m, dc*128+p]
        wT = np.ascontiguousarray(W[sl, :].T)            # [D, 128]
        return np.ascontiguousarray(
            wT.reshape(DCH, P, P).transpose(1, 0, 2)).astype(bf16)

    # maskT packed as [p, uc, b] -> [128, 32]
    mask_pk = maskT.reshape(DCH, P, B).transpose(1, 0, 2).reshape(P, DCH * B)
    in_maps = []
    for c in range(NCORES):
        sl = slice(P * c, P * (c + 1))
        misc = np.empty((P, 2 + P + DCH * B), dtype=np.float32)
        misc[:, 0] = bq[sl]
        misc[:, 1] = bk[sl]
        misc[:, 2:2 + P] = np.broadcast_to(bv[sl][None, :], (P, P))
        misc[:, 2 + P:] = mask_pk
        in_maps.append({
            "hT": hT5,
            "wq": w_pack(Wq, sl),
            "wk": w_pack(Wk, sl),
            "wv": w_pack(Wv, sl),
            "misc": misc,
        })
    return in_maps


def kernel(hidden_states, attention_mask, Wq, bq, Wk, bk, Wv, bv):
    global last_exec_time_ns, last_results
    from concourse.bass_utils import run_bass_kernel_spmd

    nc = _get_nc()
    in_maps = prepare_in_maps(hidden_states, attention_mask,
                              Wq, bq, Wk, bk, Wv, bv)

    trace = bool(int(os.environ.get("KERNEL_TRACE", "0")))
    tmpdir = os.environ.get("KERNEL_TRACE_DIR") or None
    res = run_bass_kernel_spmd(nc, in_maps, core_ids=list(range(NCORES)),
                               trace=trace, tmpdir=tmpdir)
    last_exec_time_ns = res.exec_time_ns
    last_results = res

    # gather: per-core out [B, HPC, DH+1, S]; row 0 = softmax denominator
    outs = np.stack([np.asarray(res.results[c]["out"]) for c in range(NCORES)],
                    axis=0)                             # [C, B, HPC, DH+1, S]
    ctx = outs[:, :, :, 1:, :] / outs[:, :, :, 0:1, :]  # [C, B, HPC, DH, S]
    full = ctx.transpose(4, 1, 0, 2, 3).reshape(S, B, D)   # s, b, (c, hl, j)
    return np.ascontiguousarray(full.astype(np.float32))


# revision 3
# speedup vs baseline: 1.2690x; 1.2690x over previous
"""BertSelfAttention Trainium2 kernel.

Shapes: hidden_states [S=1024, B=4, D=1024], H=16 heads of DH=64.
Sharding: 2 heads per core (8 cores). Each core receives the full hidden
states (pre-transposed + bf16-cast on host) and a 128-row slice of each
projection weight, computes the full attention chain for its two heads with
no cross-core communication, and writes unnormalized ctx^T plus the softmax
denominator per (batch, head); the final division happens on host.

Device-side layout tricks:
  - scores are computed transposed (scoresT[u, t] = q_t . k_u) so the
    additive attention mask (per key position u) is a per-partition bias
    that fuses into the Exp activation: probsT = exp(scores/8 + mask).
    Both heads' K=64 score matmuls are emitted adjacently into one PSUM
    tile [128, 2, 512] so they row-pack into the PE array (row groups
    0-63 / 64-127) and run concurrently; one ACT Exp per (u-tile, c2)
    covers both heads ([128, 1024]).
  - V carries a prepended ones-column, so the AV matmul produces the
    softmax denominator in row 0 of ctxT for free.
  - inputs are host-relaid so every DMA has large contiguous rows on both
    sides; the batch-0 load is split across both HWDGE queues (SP / ACT)
    in dc-pair pieces that the projection prologue chases; the scalar
    queue finishes early so the ACT engine is exp-only afterwards.
"""

import os
import numpy as np
import ml_dtypes

S, B, D, H = 1024, 4, 1024, 16
DH = D // H          # 64
NCORES = 8
HPC = H // NCORES    # heads per core = 2
P = 128              # partitions / d-tile / u-tile
DCH = D // P         # 8 contraction tiles
BS = B * S           # 4096 flattened (b, s)
CH = 512             # matmul free-dim chunk (PSUM bank limit for fp32)

_compiled_nc = None
last_exec_time_ns = None
last_results = None


def _build():
    import concourse.bacc as bacc
    import concourse.mybir as mybir
    import concourse.tile as tile
    from contextlib import ExitStack

    f32 = mybir.dt.float32
    bf16 = mybir.dt.bfloat16
    AF = mybir.ActivationFunctionType

    nc = bacc.Bacc("TRN2", target_bir_lowering=False, debug=False,
                   num_devices=NCORES)

    # host-relaid inputs: hT [q, p, dc, s], weights [p, dc, m]
    hT_d = nc.dram_tensor("hT", [B, P, DCH, S], bf16, kind="ExternalInput")
    wq_d = nc.dram_tensor("wq", [P, DCH, P], bf16, kind="ExternalInput")
    wk_d = nc.dram_tensor("wk", [P, DCH, P], bf16, kind="ExternalInput")
    wv_d = nc.dram_tensor("wv", [P, DCH, P], bf16, kind="ExternalInput")
    # packed per-partition constants: [bq | bk | bvb(128) | maskT(8*4)]
    misc_d = nc.dram_tensor("misc", [P, 2 + P + DCH * B], f32,
                            kind="ExternalInput")
    # row 0 of the DH+1 dim is the softmax denominator (normalized on host)
    out_d = nc.dram_tensor("out", [B, HPC, DH + 1, S], f32,
                           kind="ExternalOutput")

    with tile.TileContext(nc) as tc, ExitStack() as ctx:
        persist = ctx.enter_context(tc.tile_pool(name="persist", bufs=1))
        probs_pool = ctx.enter_context(tc.tile_pool(name="probs", bufs=34))
        out_pool = ctx.enter_context(tc.tile_pool(name="outp", bufs=4))
        ps_sc = ctx.enter_context(tc.tile_pool(name="ps_sc", bufs=2, space="PSUM"))
        ps_mm = ctx.enter_context(tc.tile_pool(name="ps_mm", bufs=2, space="PSUM"))
        ps_ctx = ctx.enter_context(tc.tile_pool(name="ps_ctx", bufs=2, space="PSUM"))

        # ---- persistent SBUF tensors ----
        # hT batch-major so batch-sized DMA pieces are dst-contiguous
        hT_sb = persist.tile([P, B, DCH, S], bf16)      # hidden^T, d-tiled
        wq_sb = persist.tile([P, DCH, P], bf16)
        wk_sb = persist.tile([P, DCH, P], bf16)
        wv_sb = persist.tile([P, DCH, P], bf16)
        misc_sb = persist.tile([P, 2 + P + DCH * B], f32)
        qT_sb = persist.tile([P, BS], bf16)             # Q^T [i, t]
        kT_sb = persist.tile([P, BS], bf16)             # K^T [i, t]
        # V in [t, j] layout + ones column per head: [t-part, t-tile, head, DH+1]
        v_sb = persist.tile([P, BS // P, HPC, DH + 1], bf16)
        dummy_sb = persist.tile([P, 256], bf16)

        bq_sb = misc_sb[:, 0:1]
        bk_sb = misc_sb[:, 1:2]
        bvb_sb = misc_sb[:, 2:2 + P]

        def mask_bias(uc, bi):
            c = 2 + P + uc * B + bi
            return misc_sb[:, c:c + 1]

        # ---- HAM warmup: dead matmuls keep the PE busy until the first
        # hidden-state piece lands, so the real work runs at 2.4 GHz.
        nc.gpsimd.memset(dummy_sb[:], 0.0)
        for _ in range(7):
            d_ps = ps_mm.tile([P, 256], f32, tag="mm", name="d_ps")
            nc.tensor.matmul(d_ps[:], dummy_sb[:, 0:P], dummy_sb[:],
                             start=True, stop=True)

        # ---- input DMAs ----
        # Two HWDGE queues (SP + ACT). Batch-0 dc-pair pieces first, split
        # across both queues so the projection prologue chases them; the
        # scalar queue finishes by ~10us so ACT is exp-only afterwards.
        def hT_piece(q, dc0, ndc, eng):
            eng.dma_start(hT_sb[:, q, dc0:dc0 + ndc, :],
                          hT_d.ap()[q, :, dc0:dc0 + ndc, :])

        nc.sync.dma_start(wk_sb[:], wk_d.ap())
        nc.scalar.dma_start(wq_sb[:], wq_d.ap())
        hT_piece(0, 0, 2, nc.sync)
        hT_piece(0, 4, 2, nc.scalar)
        hT_piece(0, 2, 2, nc.sync)
        hT_piece(0, 6, 2, nc.scalar)
        nc.sync.dma_start(misc_sb[:], misc_d.ap())
        nc.sync.dma_start(wv_sb[:], wv_d.ap())
        hT_piece(1, 0, DCH, nc.sync)
        hT_piece(2, 0, DCH, nc.sync)
        hT_piece(3, 0, DCH, nc.sync)

        nc.vector.memset(v_sb[:, :, :, 0:1], 1.0)

        scale = 1.0 / float(np.sqrt(DH))

        def emit_qk_chunk(w_sb, b_sb, dst, ci):
            bi, c2 = divmod(ci, 2)
            csl = slice(c2 * CH, (c2 + 1) * CH)
            qk_ps = ps_mm.tile([P, CH], f32, tag="mm", name="qk_ps")
            for dc in range(DCH):
                nc.tensor.matmul(
                    qk_ps[:], w_sb[:, dc, :], hT_sb[:, bi, dc, csl],
                    start=(dc == 0), stop=(dc == DCH - 1))
            nc.vector.tensor_scalar_add(dst[:, ci * CH:(ci + 1) * CH],
                                        qk_ps[:], b_sb[:])

        def emit_v_tile(tt):
            bi, ttw = divmod(tt, DCH)
            tsl = slice(ttw * P, (ttw + 1) * P)
            v_ps = ps_mm.tile([P, P], f32, tag="mm", name="v_ps")
            for dc in range(DCH):
                nc.tensor.matmul(
                    v_ps[:], hT_sb[:, bi, dc, tsl], wv_sb[:, dc, :],
                    start=(dc == 0), stop=(dc == DCH - 1))
            nc.vector.tensor_add(
                v_sb[:, tt, 0:HPC, 1:DH + 1],
                v_ps[:].rearrange("p (h j) -> p h j", j=DH),
                bvb_sb[:].rearrange("p (h j) -> p h j", j=DH))

        def qk_thunks(bi):
            th = []
            for w_sb, b_sb, dst in ((wq_sb, bq_sb, qT_sb), (wk_sb, bk_sb, kT_sb)):
                for ci in range(2 * bi, 2 * bi + 2):
                    th.append(lambda w=w_sb, b=b_sb, d=dst, c=ci:
                              emit_qk_chunk(w, b, d, c))
            return th

        def v_thunks(bi):
            return [lambda t=tt: emit_v_tile(t)
                    for tt in range(8 * bi, 8 * bi + 8)]

        def emit_av_mm(bi, hl, pps, ctx_tiles, uc):
            for c2 in range(2):
                nc.tensor.matmul(
                    ctx_tiles[c2][:],
                    v_sb[:, bi * 8 + uc, hl, :],
                    pps[uc][c2][:, hl, :],
                    start=(uc == 0), stop=(uc == DCH - 1))

        def emit_out(bi, hl, ctx_tiles):
            # ctx_ps row 0 = denominator, rows 1..DH = unnormalized ctx^T.
            # Ship both; host does the division.
            for c2 in range(2):
                csl = slice(c2 * CH, (c2 + 1) * CH)
                o_sb = out_pool.tile([DH + 1, CH], f32, name="o_sb")
                nc.vector.tensor_copy(o_sb[:], ctx_tiles[c2][:])
                nc.sync.dma_start(out_d.ap()[bi, hl, :, csl], o_sb[:])

        def new_ctx_tiles(pool=None, tag="ctx"):
            pool = pool or ps_ctx
            return [pool.tile([DH + 1, CH], f32, tag=tag, name="ctx_ps")
                    for _ in range(2)]

        # ---- prologue: batch 0's Q/K projections, dc-major in queue-arrival
        # order (sync carries dc0-3, scalar dc4-7) so the four PSUM
        # accumulation groups chase the DMA pieces. K's first chunk leads so
        # the first score tiles can fire before the last group completes.
        pro_specs = [(wk_sb, bk_sb, kT_sb, 0), (wq_sb, bq_sb, qT_sb, 0),
                     (wq_sb, bq_sb, qT_sb, 1), (wk_sb, bk_sb, kT_sb, 1)]
        pro1 = ps_sc.tile([P, HPC, CH], f32, tag="sc", name="pro1")
        pro2 = ps_sc.tile([P, HPC, CH], f32, tag="sc", name="pro2")
        pro_dst = [pro1[:, 0, :], pro1[:, 1, :], pro2[:, 0, :], pro2[:, 1, :]]
        for dc in (0, 1, 4, 5, 2, 3, 6, 7):
            for g, (w_sb, b_sb, dst, ci) in enumerate(pro_specs):
                nc.tensor.matmul(
                    pro_dst[g], w_sb[:, dc, :],
                    hT_sb[:, 0, dc, ci * CH:(ci + 1) * CH],
                    start=(dc == 0), stop=(dc == 7))
        for g, (w_sb, b_sb, dst, ci) in enumerate(pro_specs):
            osl = slice(ci * CH, (ci + 1) * CH)
            nc.vector.tensor_scalar_add(dst[:, osl], pro_dst[g], b_sb[:])

        # ---- software pipeline over batches. Per batch: scores+exp for
        # both heads per (u-tile, c2), AV of the previous batch woven in,
        # plus this batch's V projection and the next batch's Q/K
        # projection to fill the PE slack.
        prev = None          # (bi, pps)
        queue = []
        for bi in range(B):
            while queue:
                queue.pop(0)()
            queue = v_thunks(bi)
            if bi + 1 < B:
                queue += qk_thunks(bi + 1)
            pps = []
            ctxA = new_ctx_tiles() if prev is not None else None
            ctxB = None
            # h1 AV of the previous batch woven into steps 2-4 (its ctx
            # tiles borrow mm-pool slots; finishing by step 4 releases them
            # before the late projection thunks need them)
            h1_sched = {2: (0, 3), 3: (3, 6), 4: (6, 8)}
            for uc in range(DCH):
                usl = slice(bi * S + uc * P, bi * S + (uc + 1) * P)
                pp_pair = []
                for c2 in range(2):
                    qsl = slice(bi * S + c2 * CH, bi * S + (c2 + 1) * CH)
                    sc = ps_sc.tile([P, HPC, CH], f32, tag="sc", name="sc")
                    for hl in range(HPC):
                        hsl = slice(hl * DH, (hl + 1) * DH)
                        nc.tensor.matmul(
                            sc[:, hl, :], kT_sb[hsl, usl], qT_sb[hsl, qsl],
                            start=True, stop=True)
                    pp = probs_pool.tile([P, HPC, CH], bf16, name="pp")
                    nc.scalar.activation(
                        pp.rearrange("p h s -> p (h s)"),
                        sc.rearrange("p h s -> p (h s)"),
                        AF.Exp, bias=mask_bias(uc, bi), scale=scale)
                    pp_pair.append(pp)
                pps.append(pp_pair)
                if prev is not None:
                    emit_av_mm(prev[0], 0, prev[1], ctxA, uc)
                    if uc in h1_sched:
                        if ctxB is None:
                            ctxB = new_ctx_tiles(ps_mm, "mm")
                        for u2 in range(*h1_sched[uc]):
                            emit_av_mm(prev[0], 1, prev[1], ctxB, u2)
                        if h1_sched[uc][1] == DCH:
                            emit_out(prev[0], 1, ctxB)
                for _ in range(2 if uc < 4 else 1):
                    if queue:
                        queue.pop(0)()
            if prev is not None:
                emit_out(prev[0], 0, ctxA)
            prev = (bi, pps)
        # epilogue: last batch's attention output
        while queue:
            queue.pop(0)()
        pbi, ppps = prev
        ctxE0 = new_ctx_tiles()
        for uc in range(DCH):
            emit_av_mm(pbi, 0, ppps, ctxE0, uc)
        ctxE1 = new_ctx_tiles(ps_mm, "mm")
        for uc in range(DCH):
            emit_av_mm(pbi, 1, ppps, ctxE1, uc)
        emit_out(pbi, 0, ctxE0)
        emit_out(pbi, 1, ctxE1)

    nc.compile()
    return nc


def _get_nc():
    global _compiled_nc
    if _compiled_nc is None:
        _compiled_nc = _build()
    return _compiled_nc


def prepare_in_maps(hidden_states, attention_mask, Wq, bq, Wk, bk, Wv, bv):
    bf16 = ml_dtypes.bfloat16

    hs = np.asarray(hidden_states, dtype=np.float32)            # [S, B, D]
    # hT5[q, p, dc, s] = hs[s, q, dc*128 + p]
    hT5 = np.ascontiguousarray(
        hs.transpose(1, 2, 0).reshape(B, DCH, P, S).transpose(0, 2, 1, 3)
    ).astype(bf16)
    maskT = np.ascontiguousarray(
        np.asarray(attention_mask, dtype=np.float32).reshape(B, S).T)
    Wq = np.asarray(Wq, dtype=np.float32)
    Wk = np.asarray(Wk, dtype=np.float32)
    Wv = np.asarray(Wv, dtype=np.float32)
    bq = np.asarray(bq, dtype=np.float32)
    bk = np.asarray(bk, dtype=np.float32)
    bv = np.asarray(bv, dtype=np.float32)

    def w_pack(W, sl):
        # [p, dc, m] with element = W[sl][m, dc*128+p]
        wT = np.ascontiguousarray(W[sl, :].T)            # [D, 128]
        return np.ascontiguousarray(
            wT.reshape(DCH, P, P).transpose(1, 0, 2)).astype(bf16)

    # maskT packed as [p, uc, b] -> [128, 32]
    mask_pk = maskT.reshape(DCH, P, B).transpose(1, 0, 2).reshape(P, DCH * B)
    in_maps = []
    for c in range(NCORES):
        sl = slice(P * c, P * (c + 1))
        misc = np.empty((P, 2 + P + DCH * B), dtype=np.float32)
        misc[:, 0] = bq[sl]
        misc[:, 1] = bk[sl]
        misc[:, 2:2 + P] = np.broadcast_to(bv[sl][None, :], (P, P))
        misc[:, 2 + P:] = mask_pk
        in_maps.append({
            "hT": hT5,
            "wq": w_pack(Wq, sl),
            "wk": w_pack(Wk, sl),
            "wv": w_pack(Wv, sl),
            "misc": misc,
        })
    return in_maps


def kernel(hidden_states, attention_mask, Wq, bq, Wk, bk, Wv, bv):
    global last_exec_time_ns, last_results
    from concourse.bass_utils import run_bass_kernel_spmd

    nc = _get_nc()
    in_maps = prepare_in_maps(hidden_states, attention_mask,
                              Wq, bq, Wk, bk, Wv, bv)

    trace = bool(int(os.environ.get("KERNEL_TRACE", "0")))
    tmpdir = os.environ.get("KERNEL_TRACE_DIR") or None
    res = run_bass_kernel_spmd(nc, in_maps, core_ids=list(range(NCORES)),
                               trace=trace, tmpdir=tmpdir)
    last_exec_time_ns = res.exec_time_ns
    last_results = res

    # gather: per-core out [B, HPC, DH+1, S]; row 0 = softmax denominator
    outs = np.stack([np.asarray(res.results[c]["out"]) for c in range(NCORES)],
                    axis=0)                             # [C, B, HPC, DH+1, S]
    ctx = outs[:, :, :, 1:, :] / outs[:, :, :, 0:1, :]  # [C, B, HPC, DH, S]
    full = ctx.transpose(4, 1, 0, 2, 3).reshape(S, B, D)   # s, b, (c, hl, j)
    return np.ascontiguousarray(full.astype(np.float32))
